# revision 27
# baseline (speedup 1.0000x reference)
"""HTSubTree forward as a distributed Bass kernel on 8 TRN2 NeuronCores.

out[b,u,v,r] = sum_{i,j,p} x[b,(i,j)] * WL[i,u,p] * WR2[j,v,p,r]
  where WL = f0*f1*c_left (left leaf pair + core) and
        WR2 = f2*f3*c_right*c_root, both precontracted on host (tiny).
Pure batch data-parallelism: 64 of 512 batch elements per core.

v2 pipeline, per "th" step (2 pairs = 4 batch elements, 16 steps/core):
  stage1 (x2, concurrent row-tiles): psum_y[(b2,j), (pair,par,c,u)]
      pair q at PE rows q*64..q*64+63 (tile_position auto from base
      partition); both pairs' outputs land in one 2-bank PSUM tile.
  relayout (x4 copies, 1 DVE + 3 ACT): y2[(par,j), (c,pair,b2,u)] bf16
  stage2 (x8 accum, K=128): psum_o[(b2,u), (pair,v,r)] fp32
  evac (DVE): ot bf16 [128, 1024]; out DMA 2x128KB bf16 per step.
Stage2 of step th-1 is emitted between stage1(th) and its relayout so
the PE never waits on the DVE/ACT relayout.  Output is bf16 (error
~0.3% << 2e-2 budget), converted to fp32 on host.
"""

import sys

sys.path.insert(0, "/opt/trn_rl_repo")

import numpy as np

import concourse.bass as bass
import concourse.tile as tile
from concourse import bacc, mybir
from concourse.bass_utils import run_bass_kernel_spmd

NCORES = 8
B = 512
BLOC = B // NCORES  # 64 batch elements per core
NTH = BLOC // 4     # 16 steps, 4 batch elements (2 pairs) each
F32 = mybir.dt.float32
F32R = mybir.dt.float32r
BF16 = mybir.dt.bfloat16

_COMPILED = None


def _build(reps=1):
    nc = bacc.Bacc("TRN2", target_bir_lowering=False, debug=False)
    # x: partitions (q,i), free (th, b2, j)
    x_ap = nc.dram_tensor("x", [128, NTH * 128], BF16, kind="ExternalInput").ap()
    # wlf2: WL duplicated on both partition halves; free = par*256 + c*64 + u
    wlf_ap = nc.dram_tensor("wlf", [128, 512], BF16, kind="ExternalInput").ap()
    # wr2c: [c][par*64+j][v*8+r] bf16
    wr2c_ap = nc.dram_tensor("wr2c", [4, 128, 512], BF16, kind="ExternalInput").ap()
    out_ap = nc.dram_tensor("out", [BLOC * 64, 512], BF16, kind="ExternalOutput").ap()

    with tile.TileContext(nc) as tc:
        with (
            tc.tile_pool(name="weights", bufs=1) as wpool,
            tc.tile_pool(name="xin", bufs=3) as xpool,
            tc.tile_pool(name="y2", bufs=3) as ypool,
            tc.tile_pool(name="ostage", bufs=3) as opool,
            tc.tile_pool(name="py", bufs=2, space="PSUM") as pypool,
            tc.tile_pool(name="po", bufs=2, space="PSUM") as popool,
        ):
          for _rep in range(reps):
            # prefetch the scalar-engine activation table during the boot
            # preamble (otherwise it lazily loads right before the first
            # relayout copy, ~1.3us on the critical path)
            scratch = wpool.tile([128, 8], BF16, tag="scratch")
            nc.vector.memset(scratch[:, 0:4], 0)
            nc.scalar.copy(scratch[:, 4:8], scratch[:, 0:4])

            # startup DMAs split between the scalar queue (free ~2us before
            # sync) and the sync queue, so stage1 inputs land ASAP
            wlf = wpool.tile([128, 512], BF16, tag="wlf")
            nc.scalar.dma_start(wlf[:], wlf_ap[:])
            xt01 = []
            for th0 in range(2):
                t = xpool.tile([128, 128], BF16, tag="xg")
                nc.scalar.dma_start(t[:], x_ap[:, th0 * 128:(th0 + 1) * 128])
                xt01.append(t)
            wr2 = []
            for h in range(2):
                t = wpool.tile([128, 1024], BF16, tag=f"wr2c{h}")
                nc.sync.dma_start(
                    t.rearrange("p (c f) -> p c f", c=2, f=512),
                    wr2c_ap.rearrange("c p f -> p c f")[:, 2 * h:2 * h + 2],
                )
                wr2.append(t)
            wr2 = [wr2[0][:, 0:512], wr2[0][:, 512:1024],
                   wr2[1][:, 0:512], wr2[1][:, 512:1024]]

            state = None  # (y2, po, ot, th) of the previous step
            for th in range(NTH):
                if th < 2:
                    xt = xt01[th]
                else:
                    xt = xpool.tile([128, 128], BF16, tag="xg")
                    nc.sync.dma_start(xt[:], x_ap[:, th * 128:(th + 1) * 128])

                # stage1: two concurrent row-tiles (q=0 rows 0:64, q=1 rows 64:128)
                py = pypool.tile([128, 1024], F32, tag="py", space="PSUM")
                for q in range(2):
                    nc.tensor.matmul(
                        py[:, q * 512:(q + 1) * 512],
                        xt[q * 64:(q + 1) * 64, :],
                        wlf[q * 64:(q + 1) * 64, :],
                        start=True, stop=True,
                    )

                # relayout: psum_y[(b2,j), (q,par,c,u)] -> y2[(par,j), (c,q,b2,u)]
                # split by c-half across engines: DVE writes bytes [0,512),
                # ACT [512,1024) — contiguous disjoint ranges, because the
                # overlap tracker works on byte ranges (ignoring partitions)
                # and any interleaved split falsely serializes the engines
                y2 = ypool.tile([128, 1024], BF16, tag="y2")
                src_v = py.rearrange("(b2 j) (q par ch cl u) -> ch b2 par j q cl u",
                                     b2=2, j=64, q=2, par=2, ch=2, cl=2, u=64)
                dst_v = y2.rearrange("(par j) (ch cl q b2 u) -> ch b2 par j q cl u",
                                     par=2, j=64, ch=2, cl=2, q=2, b2=2, u=64)
                for ch in range(2):
                    eng = nc.vector.tensor_copy if ch == 0 else nc.scalar.copy
                    for b2 in range(2):
                        for par in range(2):
                            eng(dst_v[ch][b2][par], src_v[ch][b2][par])

                # stage2 + evac + out-DMA of the previous step AFTER this
                # step's relayout, so the strict-FIFO DVE/ACT queues never
                # block a relayout behind an evac that waits on the PE
                if state is not None:
                    _stage2(nc, wr2, out_ap, *state)

                po = popool.tile([128, 1024], F32, tag="po", space="PSUM")
                ot = opool.tile([128, 1024], BF16, tag="ot")
                state = (y2, po, ot, th)

            _stage2(nc, wr2, out_ap, *state)

    nc.compile()
    return nc


def _stage2(nc, wr2, out_ap, y2, po, ot, th):
    # K=128 accumulating matmuls: lhsT = y2[:, c*256+q*128 : +128]
    for q in range(2):
        for c in range(4):
            nc.tensor.matmul(
                po[:, q * 512:(q + 1) * 512],
                y2[:, c * 256 + q * 128: c * 256 + (q + 1) * 128],
                wr2[c],
                start=(c == 0), stop=(c == 3),
            )
    nc.vector.tensor_copy(ot[:, 0:512], po[:, 0:512])
    nc.scalar.copy(ot[:, 512:1024], po[:, 512:1024])
    dst = out_ap[256 * th: 256 * th + 256, :].rearrange("(q p) f -> p q f", q=2, p=128)
    nc.sync.dma_start(dst, ot.rearrange("p (q f) -> p q f", q=2, f=512))


def _host_prep(x, factors, cores):
    """Pre-contract the tiny parameters and lay out per-core shards."""
    f0, f1, f2, f3 = factors[0], factors[1], factors[2], factors[3]
    c_root, c_left, c_right = cores[0], cores[1], cores[2]
    # WL[(i0,i1),(o0,o1),p=r02]
    wl = np.einsum("ioa,jpb,abr->ijopr", f0, f1, c_left, optimize=True)
    wl = wl.reshape(64, 64, 8)  # [i, u, p]
    # WRq[(i2,i3),(o2,o3),q=r24];  WR2[j,v,p,r] = sum_q WRq * c_root[p,q,r]
    wrq = np.einsum("ioc,jpd,cdq->ijopq", f2, f3, c_right, optimize=True).reshape(64, 64, 8)
    wr2 = np.einsum("jvq,pqr->jvpr", wrq, c_root, optimize=True)  # [j, v, p, r]

    import ml_dtypes
    # wlf [64, 512]: free = par*256 + c*64 + u  with  p = 2c + par; dup rows
    wlf1 = np.ascontiguousarray(
        wl.reshape(64, 64, 4, 2).transpose(0, 3, 2, 1).reshape(64, 512))
    wlf = np.concatenate([wlf1, wlf1], axis=0).astype(ml_dtypes.bfloat16)
    # wr2c [4, 128, 512]: [c][par*64+j][v*8+r] = wr2[j, v, 2c+par, r]
    wr2c = np.ascontiguousarray(
        wr2.transpose(2, 0, 1, 3).reshape(4, 2, 64, 64, 8).reshape(4, 128, 512)
    ).astype(ml_dtypes.bfloat16)

    xf = x.reshape(B, 64, 64).astype(ml_dtypes.bfloat16)
    xs = []
    for core in range(NCORES):
        xl = xf[core * BLOC:(core + 1) * BLOC]  # [64(b), 64(i), 64(j)]
        # [th, q, b2, i, j] -> [q, i, th, b2, j]
        xr = xl.reshape(NTH, 2, 2, 64, 64).transpose(1, 3, 0, 2, 4)
        xs.append(np.ascontiguousarray(xr.reshape(128, NTH * 128)))
    return xs, wlf, wr2c


def kernel(x, factors, cores, _want_profile=False):
    global _COMPILED
    x = np.asarray(x, dtype=np.float32)
    factors = np.asarray(factors, dtype=np.float32)
    cores = np.asarray(cores, dtype=np.float32)
    if _COMPILED is None:
        _COMPILED = _build()
    nc = _COMPILED
    xs, wlf, wr2c = _host_prep(x, factors, cores)
    in_maps = [{"x": xs[c], "wlf": wlf, "wr2c": wr2c} for c in range(NCORES)]
    res = run_bass_kernel_spmd(nc, in_maps, list(range(NCORES)), trace=_want_profile)
    out = np.concatenate(
        [res.results[c]["out"].astype(np.float32).reshape(BLOC, 8, 8, 8, 8, 8)
         for c in range(NCORES)]
    )
    if _want_profile:
        return out, res
    return out
